# revision 5
# baseline (speedup 1.0000x reference)
"""2-layer GCN (gnn_message_passing) on 8 Trainium2 NeuronCores — v3.

Per-core strategy:
  - Phase A: h = x@W1 with fp8e3m4 x (halves the 98MB x stream; rel-err
    checked ~8.5e-3 total) on PE; h cast to fp8e3m4, AllGathered in 3
    chunks (last chunk smallest so the final AG lands early).
  - Layer 1 runs in 3 waves (one per h chunk): wave s aggregates every
    dst block's stream-s edges as soon as AllGather s completes,
    accumulating into a bf16 SBUF staging table. One-hot M tiles use 64
    dst columns (halves M bytes) with PSUM partition-sliced accumulation
    (bases 0/64). Wave 2 adds self loop + bias, relu, zw = z@W2 ->
    local zwR table.
  - Layer 2: src-partitioned; gather zw rows from LOCAL zwR, aggregate
    per 64-wide global dst block into [128,8] psum pairs, stage per
    dst-core, ReduceScatter(add) of [NC*128, B, O] bf16. Final out =
    RS + precomputed (self loop + b2).
"""
import sys
sys.path.insert(0, "/opt/trn_rl_repo")
import numpy as np
import ml_dtypes

N = 50000
NPAD = 50176
NC = 8
PC = NPAD // NC          # 6272
B = PC // 128            # 49 dst blocks per core
GB64 = NPAD // 64        # 784 global 64-wide dst blocks
KDIM = 7688
KPAD = 7808
KO = KPAD // 128         # 61
H = 200
HPAD = 256
O = 8
CH = [2688, 3584]        # h chunks; AG of chunk 0 hides under phase A
CB = [0, 2688]
BSPA = 21                # chunk-0 block boundary (2688 rows)
NS = 2
L2G = 14                 # 64-blocks per L2 chunk (98 per dst core = 7 chunks)

BF16 = ml_dtypes.bfloat16
FP8 = ml_dtypes.float8_e3m4


def _wrap_idx(grp):
    return np.tile(grp.reshape(-1, 16).T, (8, 1)).astype(np.int16)


def _preprocess(x, edge_weight, W1, b1, W2, b2, edge_index):
    x = np.asarray(x, dtype=np.float32)
    edge_weight = np.asarray(edge_weight, dtype=np.float32)
    W1 = np.asarray(W1, dtype=np.float32)
    b1 = np.asarray(b1, dtype=np.float32)
    W2 = np.asarray(W2, dtype=np.float32)
    b2 = np.asarray(b2, dtype=np.float32)
    src = np.asarray(edge_index[0], dtype=np.int64)
    dst = np.asarray(edge_index[1], dtype=np.int64)

    deg = np.bincount(dst, weights=edge_weight.astype(np.float64), minlength=N)
    deg += 1.0
    dis = np.where(deg > 0, deg ** -0.5, 0.0).astype(np.float32)
    norm = dis[src] * edge_weight * dis[dst]
    dis2 = (dis * dis).astype(np.float32)

    # ---------- Layer 1: dst-partitioned; (block, stream, half) groups ----
    core1 = dst // PC
    block1 = (dst % PC) // 128
    q1 = (dst % 128) // 64
    dl1 = dst % 64
    srcloc = src % PC
    stream = (srcloc >= CB[1]).astype(np.int64)

    key1 = (((core1 * B + block1) * NS + stream) * 2 + q1)
    o1 = np.argsort(key1, kind="stable")
    src1, norm1, dl1s, key1s = src[o1], norm[o1], dl1[o1], key1[o1]
    cnt1 = np.bincount(key1s, minlength=NC * B * NS * 2).reshape(NC, B, NS, 2)
    st1 = np.zeros(NC * B * NS * 2 + 1, dtype=np.int64)
    np.cumsum(cnt1.ravel(), out=st1[1:])

    T1 = np.maximum(1, -(-cnt1.max(axis=0) // 128))   # [B, NS, 2]
    # tile order: block-major (b, s, q) so Mb slices stay contiguous
    order = [(b, s, q) for b in range(B) for s in range(NS) for q in range(2)]
    col = {}
    off = 0
    for (b, s, q) in order:
        col[(s, b, q)] = off
        off += int(T1[b, s, q])
    TSUM1 = off

    # ---------- Layer 2: src-partitioned; 64-wide global dst groups ------
    core2 = src // PC
    g64 = dst // 64
    dl2 = dst % 64
    key2 = core2 * GB64 + g64
    o2 = np.argsort(key2, kind="stable")
    src2, norm2, dl2s, key2s = src[o2], norm[o2], dl2[o2], key2[o2]
    cnt2 = np.bincount(key2s, minlength=NC * GB64).reshape(NC, GB64)
    st2 = np.zeros(NC * GB64 + 1, dtype=np.int64)
    np.cumsum(cnt2.ravel(), out=st2[1:])
    T2 = np.maximum(1, -(-cnt2.max(axis=0) // 128))   # [GB64]
    TSUM2 = int(T2.sum())
    col2 = np.zeros(GB64, dtype=np.int64)
    np.cumsum(T2[:-1], out=col2[1:])

    # ---------- shared host tensors ----------
    x_pad = np.zeros((NPAD, KPAD), dtype=np.float32)
    x_pad[:N, :KDIM] = x
    W1_pad = np.zeros((KPAD, H), dtype=np.float32)
    W1_pad[:KDIM] = W1
    W1_pre = np.ascontiguousarray(
        W1_pad.reshape(KO, 128, H).transpose(1, 0, 2)).astype(BF16)
    b1_rep = np.tile(b1[None, :], (128, 1)).astype(np.float32)
    b2_rep = np.tile(b2[None, :], (128, 1)).astype(np.float32)
    W2_pre = np.zeros((128, 2, O), dtype=np.float32)
    W2_pre[:128, 0, :] = W2[:128]
    W2_pre[: H - 128, 1, :] = W2[128:H]
    dis2_pad = np.zeros(NPAD, dtype=np.float32)
    dis2_pad[:N] = dis2

    per_core = []
    for r in range(NC):
        xr = x_pad[r * PC:(r + 1) * PC]
        x_pre = np.ascontiguousarray(
            xr.reshape(B, 128, KO, 128).transpose(0, 3, 2, 1)
        ).reshape(B, 128, KPAD).astype(FP8)

        idx1_cols = np.zeros((128, TSUM1 * 8), dtype=np.int16)
        m1 = np.zeros((TSUM1 * 128, 64), np.float32)
        for (b, s, q) in order:
            gkey = ((r * B + b) * NS + s) * 2 + q
            lo_i, hi_i = st1[gkey], st1[gkey + 1]
            n = int(hi_i - lo_i)
            t0 = col[(s, b, q)]
            Tn = int(T1[b, s, q])
            sseg = src1[lo_i:hi_i]
            ids = np.zeros(Tn * 128, dtype=np.int64)
            ids[:n] = (sseg // PC) * CH[s] + (sseg % PC - CB[s])
            idx1_cols[:, t0 * 8:(t0 + Tn) * 8] = _wrap_idx(ids)
            rows = np.arange(t0 * 128, t0 * 128 + n)
            m1[rows, dl1s[lo_i:hi_i]] = norm1[lo_i:hi_i]
        m1 = np.ascontiguousarray(
            m1.reshape(TSUM1, 128, 64).transpose(1, 0, 2)).astype(BF16)

        idx2_cols = np.zeros((128, TSUM2 * 8), dtype=np.int16)
        m2 = np.zeros((TSUM2 * 128, 64), np.float32)
        for g in range(GB64):
            gkey = r * GB64 + g
            lo_i, hi_i = st2[gkey], st2[gkey + 1]
            n = int(hi_i - lo_i)
            t0 = int(col2[g])
            Tn = int(T2[g])
            ids = np.zeros(Tn * 128, dtype=np.int64)
            ids[:n] = src2[lo_i:hi_i] % PC
            idx2_cols[:, t0 * 8:(t0 + Tn) * 8] = _wrap_idx(ids)
            rows = np.arange(t0 * 128, t0 * 128 + n)
            m2[rows, dl2s[lo_i:hi_i]] = norm2[lo_i:hi_i]
        m2 = np.ascontiguousarray(
            m2.reshape(TSUM2, 128, 64).transpose(1, 0, 2)).astype(BF16)

        dis2_blk = dis2_pad[r * PC:(r + 1) * PC].reshape(B, 128).T.copy()

        per_core.append({
            "x_pre": x_pre, "w1": W1_pre, "b1r": b1_rep, "b2r": b2_rep,
            "w2": W2_pre, "idx1": idx1_cols, "idx2": idx2_cols,
            "dis2": dis2_blk, "m1": m1, "m2": m2,
        })

    meta = {
        "T1": T1.tolist(), "col1": {f"{s}_{b}_{q}": v for (s, b, q), v
                                    in col.items()},
        "TSUM1": TSUM1,
        "T2": [int(v) for v in T2], "col2": [int(v) for v in col2],
        "TSUM2": TSUM2,
    }
    return per_core, meta


def _build_program(meta):
    import concourse.bass as bass
    import concourse.bacc as bacc
    import concourse.mybir as mybir
    import concourse.tile as tile

    T1, TSUM1 = meta["T1"], meta["TSUM1"]
    colmap = meta["col1"]
    T2, col2, TSUM2 = meta["T2"], meta["col2"], meta["TSUM2"]

    def col1(s, b, q):
        return colmap[f"{s}_{b}_{q}"]

    nc = bacc.Bacc("TRN2", target_bir_lowering=False, debug=False,
                   num_devices=NC)
    f32, bf16, i16 = mybir.dt.float32, mybir.dt.bfloat16, mybir.dt.int16
    f8 = mybir.dt.float8e3

    x_d = nc.dram_tensor("x_pre", [B, 128, KPAD], f8, kind="ExternalInput")
    w1_d = nc.dram_tensor("w1", [128, KO, H], bf16, kind="ExternalInput")
    b1_d = nc.dram_tensor("b1r", [128, H], f32, kind="ExternalInput")
    b2_d = nc.dram_tensor("b2r", [128, O], f32, kind="ExternalInput")
    w2_d = nc.dram_tensor("w2", [128, 2, O], f32, kind="ExternalInput")
    idx1_d = nc.dram_tensor("idx1", [128, TSUM1 * 8], i16, kind="ExternalInput")
    idx2_d = nc.dram_tensor("idx2", [128, TSUM2 * 8], i16, kind="ExternalInput")
    d2_d = nc.dram_tensor("dis2", [128, B], f32, kind="ExternalInput")
    m1_d = nc.dram_tensor("m1", [128, TSUM1, 64], bf16, kind="ExternalInput")
    m2_d = nc.dram_tensor("m2", [128, TSUM2, 64], bf16, kind="ExternalInput")
    out_d = nc.dram_tensor("out", [PC, O], f32, kind="ExternalOutput")

    with tile.TileContext(nc) as tc:
        with tc.tile_pool(name="cons", bufs=1) as cons, \
             tc.tile_pool(name="xt", bufs=2) as xtp, \
             tc.tile_pool(name="g0", bufs=4) as g0p, \
             tc.tile_pool(name="g1", bufs=4) as g1p, \
             tc.tile_pool(name="g2", bufs=4) as g2p, \
             tc.tile_pool(name="mb", bufs=3) as mbp, \
             tc.tile_pool(name="zwres", bufs=B) as zwp, \
             tc.tile_pool(name="l2", bufs=2) as l2p, \
             tc.tile_pool(name="wk", bufs=3) as wk, \
             tc.tile_pool(name="dram", bufs=1, space="DRAM") as dram, \
             tc.tile_pool(name="pA", bufs=2, space="PSUM") as pA, \
             tc.tile_pool(name="pG", bufs=2, space="PSUM") as pG, \
             tc.tile_pool(name="pT", bufs=1, space="PSUM") as pT, \
             tc.tile_pool(name="pO", bufs=2, space="PSUM") as pO:

            w1_sb = cons.tile([128, KO, H], bf16)
            nc.sync.dma_start(w1_sb[:], w1_d.ap())
            b1_sb = cons.tile([128, H], f32)
            nc.sync.dma_start(b1_sb[:], b1_d.ap())
            b2_sb = cons.tile([128, O], f32)
            nc.sync.dma_start(b2_sb[:], b2_d.ap())
            w2_sb = cons.tile([128, 2, O], f32)
            nc.sync.dma_start(w2_sb[:], w2_d.ap())
            idx1_sb = cons.tile([128, TSUM1 * 8], i16)
            nc.sync.dma_start(idx1_sb[:], idx1_d.ap())
            d2_sb = cons.tile([128, B], f32)
            nc.sync.dma_start(d2_sb[:], d2_d.ap())
            selfpre = cons.tile([128, B, O], f32)
            zwblks = {}

            hR = [dram.tile([CH[s], HPAD], f8, name=f"hR{s}")
                  for s in range(NS)]
            hfull = [dram.tile([NC * CH[s], HPAD], f8, addr_space="Shared",
                               name=f"hfull{s}")
                     for s in range(NS)]
            zwR = dram.tile([PC, 128], bf16)
            rs_in = dram.tile([NC * 128, B, O], bf16)
            rs_out = dram.tile([128, B, O], bf16)

            def hR_slice(b):
                s = 0 if b < BSPA else 1
                b0 = b - (0 if s == 0 else BSPA)
                return hR[s][b0 * 128:(b0 + 1) * 128, :], s

            # ---- phase A
            for b in range(B):
                xt = xtp.tile([128, KO, 128], f8, tag="xt")
                nc.sync.dma_start(xt[:], x_d.ap()[b])
                ph = pA.tile([128, H], f32, tag="ph",
                             padded_shape=[128, 512])
                for k in range(KO):
                    nc.tensor.matmul(ph[:], lhsT=xt[:, k, :], rhs=w1_sb[:, k, :],
                                     start=(k == 0), stop=(k == KO - 1))
                hblk = wk.tile([128, HPAD], f8, tag="hblk")
                nc.vector.memset(hblk[:, H:], 0.0)
                nc.vector.tensor_copy(hblk[:, :H], ph[:])
                sl, _ = hR_slice(b)
                nc.sync.dma_start(sl, hblk[:])
                if b == BSPA - 1:
                    nc.gpsimd.collective_compute(
                        "AllGather", mybir.AluOpType.bypass,
                        ins=[hR[0][:]], outs=[hfull[0][:]],
                        replica_groups=[list(range(NC))])

            # ---- AllGather h chunk B, then one-pass layer 1
            from concourse.masks import make_identity
            ident = cons.tile([128, 128], f32)
            make_identity(nc, ident[:])

            nc.gpsimd.collective_compute(
                "AllGather", mybir.AluOpType.bypass,
                ins=[hR[1][:]], outs=[hfull[1][:]],
                replica_groups=[list(range(NC))])

            PF = 3
            gpools = (g0p, g1p)
            gtiles = [{}, {}]

            def issue_g(b, strm):
                c0 = col1(strm, b, 0)
                Tn = T1[b][strm][0] + T1[b][strm][1]
                t = gpools[strm].tile([128, Tn, HPAD], f8, tag=f"G{strm}")
                nc.gpsimd.dma_gather(
                    t[:], hfull[strm][:, :],
                    idx1_sb[:, c0 * 8:(c0 + Tn) * 8],
                    Tn * 128, Tn * 128, HPAD, single_packet=False)
                gtiles[strm][b] = t

            for b in range(PF):
                issue_g(b, 0)
                issue_g(b, 1)
            for b in range(B):
                if b + PF < B:
                    issue_g(b + PF, 0)
                    issue_g(b + PF, 1)
                G0, G1 = gtiles[0].pop(b), gtiles[1].pop(b)
                Gs = (G0, G1)
                c0 = col1(0, b, 0)
                TTb = sum(T1[b][strm][q] for strm in range(NS) for q in range(2))
                Mb = mbp.tile([128, TTb, 64], bf16, tag="Mb")
                nc.sync.dma_start(Mb[:], m1_d.ap()[:, c0:c0 + TTb, :])
                acc = pG.tile([128, H], f32, tag="acc",
                              padded_shape=[128, 512])
                for q in range(2):
                    runs = [(strm, t) for strm in range(NS)
                            for t in range(T1[b][strm][q])]
                    for i, (strm, t) in enumerate(runs):
                        goff = (T1[b][strm][0] if q == 1 else 0) + t
                        moff = (col1(strm, b, q) - c0) + t
                        nc.tensor.matmul(
                            acc[q * 64:(q + 1) * 64, :],
                            lhsT=Mb[:, moff, :], rhs=Gs[strm][:, goff, :H],
                            start=(i == 0), stop=(i == len(runs) - 1))
                sl, _ = hR_slice(b)
                loc = wk.tile([128, HPAD], f8, tag="loc")
                nc.sync.dma_start(loc[:], sl)
                selfT = wk.tile([128, H], f32, tag="selfT")
                nc.vector.tensor_scalar(
                    out=selfT[:], in0=loc[:, :H],
                    scalar1=d2_sb[:, b:b + 1], scalar2=None,
                    op0=mybir.AluOpType.mult)
                zsum = wk.tile([128, H], f32, tag="zsum")
                nc.vector.tensor_add(out=zsum[:], in0=acc[:], in1=selfT[:])
                nc.vector.tensor_add(out=zsum[:], in0=zsum[:], in1=b1_sb[:])
                zf = wk.tile([128, H], f32, tag="zf")
                nc.scalar.activation(zf[:], zsum[:],
                                     mybir.ActivationFunctionType.Relu)
                zT = wk.tile([128, 2, 128], f32, tag="zT")
                for kt, (k0, kw) in enumerate(((0, 128), (128, H - 128))):
                    pt = pT.tile([128, 128], f32, tag="pt",
                                 padded_shape=[128, 512])
                    nc.tensor.transpose(pt[:kw, :], zf[:, k0:k0 + kw],
                                        ident[:])
                    nc.vector.tensor_copy(zT[:kw, kt, :], pt[:kw, :])
                po = pO.tile([128, O], f32, tag="po",
                             padded_shape=[128, 512])
                nc.tensor.matmul(po[:], lhsT=zT[:, 0, :],
                                 rhs=w2_sb[:, 0, :],
                                 start=True, stop=False)
                nc.tensor.matmul(po[:], lhsT=zT[:H - 128, 1, :],
                                 rhs=w2_sb[:H - 128, 1, :],
                                 start=False, stop=True)
                zwblk = zwp.tile([128, 128], bf16, tag="zwblk")
                nc.vector.memset(zwblk[:, O:], 0.0)
                nc.vector.tensor_copy(zwblk[:, :O], po[:])
                nc.sync.dma_start(zwR[b * 128:(b + 1) * 128, :],
                                  zwblk[:])
                zwblks[b] = zwblk
                selfw = wk.tile([128, O], f32, tag="selfw")
                nc.vector.tensor_scalar(
                    out=selfw[:], in0=zwblk[:, :O],
                    scalar1=d2_sb[:, b:b + 1], scalar2=None,
                    op0=mybir.AluOpType.mult)
                nc.vector.tensor_add(out=selfpre[:, b, :],
                                     in0=selfw[:], in1=b2_sb[:])

            # ---- layer 2: src-partitioned partials + ReduceScatter
            for c in range(NC):
                st = l2p.tile([128, B, O], bf16, tag="st")
                for j0 in range(0, 2 * B, L2G):
                    g0 = c * 2 * B + j0
                    c0, c1 = col2[g0], col2[g0 + L2G - 1] + T2[g0 + L2G - 1]
                    TTc = c1 - c0
                    i2 = l2p.tile([128, TTc * 8], i16, tag="i2")
                    nc.sync.dma_start(i2[:], idx2_d.ap()[:, c0 * 8:c1 * 8])
                    G2 = l2p.tile([128, TTc, 128], bf16, tag="G2")
                    nc.gpsimd.dma_gather(
                        G2[:], zwR[:, :], i2[:],
                        TTc * 128, TTc * 128, 128, single_packet=False)
                    M2 = l2p.tile([128, TTc, 64], bf16, tag="M2")
                    nc.sync.dma_start(M2[:], m2_d.ap()[:, c0:c1, :])
                    for jp in range(0, L2G, 2):   # pairs of 64-blocks
                        p2 = pO.tile([128, O], f32, tag="po",
                                     padded_shape=[128, 512])
                        for q in range(2):
                            g = g0 + jp + q
                            toff = col2[g] - c0
                            for t in range(T2[g]):
                                nc.tensor.matmul(
                                    p2[q * 64:(q + 1) * 64, :],
                                    lhsT=M2[:, toff + t, :],
                                    rhs=G2[:, toff + t, :O],
                                    start=(t == 0), stop=(t == T2[g] - 1))
                        nc.vector.tensor_copy(st[:, (j0 + jp) // 2, :], p2[:])
                nc.sync.dma_start(rs_in[c * 128:(c + 1) * 128, :, :], st[:])

            nc.gpsimd.collective_compute(
                "ReduceScatter", mybir.AluOpType.add,
                ins=[rs_in[:]], outs=[rs_out[:]],
                replica_groups=[list(range(NC))])

            # ---- final: out = RS + (self loop + b2)
            rsb = cons.tile([128, B, O], bf16)
            nc.sync.dma_start(rsb[:], rs_out[:])
            obf = cons.tile([128, B, O], f32)
            nc.vector.tensor_add(out=obf[:], in0=rsb[:], in1=selfpre[:])
            for b in range(B):
                nc.sync.dma_start(out_d.ap()[b * 128:(b + 1) * 128, :],
                                  obf[:, b, :])

    nc.compile()
    return nc


def build(inputs):
    per_core, meta = _preprocess(**inputs)
    nc = _build_program(meta)
    return nc, per_core, meta


def kernel(**inputs) -> np.ndarray:
    from concourse import bass_utils
    nc, per_core, meta = build(inputs)
    res = bass_utils.run_bass_kernel_spmd(nc, per_core, core_ids=list(range(NC)))
    out = np.concatenate([res.results[c]["out"] for c in range(NC)], axis=0)
    return np.ascontiguousarray(out[:N]).astype(np.float32)
